# revision 17
# baseline (speedup 1.0000x reference)
"""Trainium2 Bass kernel for nn_AggregationEncoder (gnn_message_passing).

Reference computation:
    adj[g, m] = 1 where an edge (g, m) exists (set semantics)
    norm[m]   = max(sum_g adj[g, m], 1)
    out[b, m, d] = sum_g adj[g, m] / norm[m] * x[b, g, d]

Structural facts hardcoded from the problem spec:
  - x: [B=2, G=40962, D=512] float32; edge_index: [E=122880, 2] int64,
    BOTH columns in [0, 2562) -> contraction only needs x[:, :2562, :].
  - M = 2562 mesh nodes.

Sharding (8 cores): 2 batches x 4 mesh-column chunks of W=642 columns.
Host work is sharding/layout only: dedup the edge set, lay it out as a
dense 0/1 fp8 adjacency chunk the device DMAs directly, pre-cast x to
bf16, precompute per-column reciprocal degrees (pure function of
edge_index).

Orientation: the kernel computes out^T[d, m] with x tiles as the
STATIONARY operand (bf16 weights) and the adjacency as the MOVING
operand in fp8e4m3 (measured full-rate as moving; fp8 *weights* run
~20% slow). This halves adjacency bytes — the kernel is DMA-wire-bound
(~235 GB/s/core aggregate) — and the ragged mesh columns 640/641 ride
along in the 130-wide column group (their senders are host-gathered
into x pad rows 2562..2687, adjacency rows restricted to k-tile 20).
Normalization is a DVE multiply against a host-replicated bf16
reciprocal row; outputs ship as bf16 transposed and the host
reassembles (error stays ~3x under the 2e-2 gate).

Per kt (21 of them): 4 d-tiles x (512+130 moving rows) = 8 matmuls into
8 PSUM banks, accumulated across all kts; final k-group is d-tile-major
so drains (DVE normalize + output DMA on alternating rings) overlap the
stream tail. Warm-up matmuls hold the PE p-state through the pre-data
window (a >1us idle gap resets the clock and costs a re-ramp).
"""

import numpy as np
import ml_dtypes

B = 2
G = 40962
D = 512
M = 2562            # mesh nodes
SEN = 2562          # senders (edge values < 2562)
GP = 2688           # padded sender rows = 21*128
KT = GP // 128      # 21 k-tiles
NQ = 4              # mesh-column chunks
W = 642             # mesh columns per chunk
WA = 512            # first moving column group
WB = W - WA         # second moving column group (incl. ragged cols)
NDT = 4             # d-tiles of 128
PAD0 = 2562         # first gather-pad row
NPAD = GP - PAD0    # 126 gather slots
N_CORES = 8

SC = 7              # input DMA chunks of PL=3 k-tiles
PL = 3

_NC_CACHE = None


def _build_bass():
    import concourse.bacc as bacc
    import concourse.mybir as mybir
    import concourse.tile as tile

    dt = mybir.dt
    nc = bacc.Bacc("TRN2", target_bir_lowering=False, debug=False,
                   num_devices=N_CORES)

    # DRAM chunk blocks: partition-major inside each 3-ktile block so
    # every DMA descriptor is one partition's contiguous run and the
    # descriptor stream is sequential in DRAM.
    xh = nc.dram_tensor("xh", [SC, 128, PL, D], dt.bfloat16,
                        kind="ExternalInput")
    adj = nc.dram_tensor("adj", [SC, 128, PL, W], dt.float8e4,
                         kind="ExternalInput")
    rb = nc.dram_tensor("rb", [128, W], dt.bfloat16, kind="ExternalInput")
    out = nc.dram_tensor("out", [D, W], dt.bfloat16, kind="ExternalOutput")

    with tile.TileContext(nc) as tc:
        with (
            tc.tile_pool(name="sbuf", bufs=1) as sb,
            tc.tile_pool(name="outb", bufs=2) as outb,
            tc.tile_pool(name="psum", bufs=1, space="PSUM") as ps,
        ):
            a_sb = sb.tile([128, KT, W], dt.float8e4)
            x_sb = sb.tile([128, KT, D], dt.bfloat16)
            rb_sb = sb.tile([128, W], dt.bfloat16)
            for s in range(SC):
                nc.sync.dma_start(out=x_sb[:, s * PL:(s + 1) * PL, :],
                                  in_=xh[s])
                nc.scalar.dma_start(out=a_sb[:, s * PL:(s + 1) * PL, :],
                                    in_=adj[s])
                if s == 1:
                    nc.scalar.dma_start(rb_sb[:], rb[:])

            pA = [ps.tile([128, WA], dt.float32, tag=f"pa{i}",
                          name=f"pa{i}") for i in range(NDT)]
            pB = [ps.tile([128, WB], dt.float32, tag=f"pb{i}",
                          name=f"pb{i}") for i in range(NDT)]

            # Warm-ups (PE p-state) — same dtype mix as the real stream;
            # they share pA[0], which kt0's start=True reset clears.
            warm_src = sb.tile([128, D], dt.bfloat16)
            nc.vector.memset(warm_src[:], 1.0)
            warm8 = sb.tile([128, WA], dt.float8e4)
            nc.vector.memset(warm8[:], 1.0)
            for _ in range(8):
                nc.tensor.matmul(pA[0][0:32, :], lhsT=warm_src[:, 0:32],
                                 rhs=warm8[:], start=True, stop=True)

            def mm(dtile, kt):
                lhs = x_sb[:, kt, dtile * 128:(dtile + 1) * 128]
                nc.tensor.matmul(pA[dtile][:, :], lhsT=lhs,
                                 rhs=a_sb[:, kt, 0:WA],
                                 start=(kt == 0), stop=(kt == KT - 1))
                nc.tensor.matmul(pB[dtile][:, :], lhsT=lhs,
                                 rhs=a_sb[:, kt, WA:W],
                                 start=(kt == 0), stop=(kt == KT - 1))

            for t in range(SC - 1):          # kts 0..17
                for kt in (3 * t, 3 * t + 1, 3 * t + 2):
                    for dtile in range(NDT):
                        mm(dtile, kt)

            # Final k-group d-tile-major: finish each d-tile's A columns
            # first so its normalize runs under the B matmuls, then B;
            # normalize+store overlap the remaining stream.
            for dtile in range(NDT):
                o_sb = outb.tile([128, W], dt.bfloat16,
                                 tag=f"osb{dtile % 2}", name=f"osb{dtile}")
                for kt in (18, 19, 20):
                    nc.tensor.matmul(pA[dtile][:, :],
                                     lhsT=x_sb[:, kt,
                                               dtile * 128:(dtile + 1) * 128],
                                     rhs=a_sb[:, kt, 0:WA],
                                     start=False, stop=(kt == KT - 1))
                nc.vector.tensor_mul(o_sb[:, 0:WA], pA[dtile][:, :],
                                     rb_sb[:, 0:WA])
                for kt in (18, 19, 20):
                    nc.tensor.matmul(pB[dtile][:, :],
                                     lhsT=x_sb[:, kt,
                                               dtile * 128:(dtile + 1) * 128],
                                     rhs=a_sb[:, kt, WA:W],
                                     start=False, stop=(kt == KT - 1))
                nc.vector.tensor_mul(o_sb[:, WA:W], pB[dtile][:, :],
                                     rb_sb[:, WA:W])
                if dtile % 2 == 0:
                    nc.sync.dma_start(
                        out[dtile * 128:(dtile + 1) * 128, :], o_sb[:])
                else:
                    nc.scalar.dma_start(
                        out[dtile * 128:(dtile + 1) * 128, :], o_sb[:])

    nc.finalize()
    return nc


def _get_nc():
    global _NC_CACHE
    if _NC_CACHE is None:
        _NC_CACHE = _build_bass()
    return _NC_CACHE


def _pm(a):
    """[GP, F] row-major -> [SC, 128, PL, F] 3-plane partition-minor."""
    return np.ascontiguousarray(
        a.reshape(SC, PL, 128, a.shape[1]).transpose(0, 2, 1, 3))


def _host_shard(grid_node_features, edge_index):
    """Dedup edges and lay them out as per-chunk dense adjacency + padded
    bf16 x + replicated reciprocal degrees. Returns per-core inputs."""
    x = np.asarray(grid_node_features)
    e = np.asarray(edge_index)
    g = e[:, 0].astype(np.int64)
    m = e[:, 1].astype(np.int64)
    key = np.unique(g * M + m)           # set semantics
    g = key // M
    m = key % M
    deg = np.bincount(m, minlength=M).astype(np.float64)
    rec_full = (1.0 / np.maximum(deg, 1.0)).astype(np.float32)

    ONE8 = np.uint8(0x38)                # fp8 e4m3 1.0

    adjs = []
    rbs = []
    glists = []
    for q in range(NQ):
        lo = q * W
        sel = (m >= lo) & (m < lo + W)
        gq = g[sel]
        mq = m[sel] - lo
        av = np.zeros((GP, W), np.uint8)
        main = mq < W - 2
        av[gq[main], mq[main]] = ONE8
        # ragged columns 640/641: senders >= 2560 sit in k-tile 20
        # already; senders < 2560 are gathered into pad rows.
        glist = []
        for r in (W - 2, W - 1):
            if lo + r >= M:
                continue
            snd = np.sort(gq[mq == r])
            av[snd[snd >= 2560], r] = ONE8
            for s in snd[snd < 2560]:
                av[PAD0 + len(glist), r] = ONE8
                glist.append(s)
        if len(glist) > NPAD:
            raise ValueError(f"gather overflow: {len(glist)} > {NPAD}")
        adjs.append(_pm(av).view(ml_dtypes.float8_e4m3))
        glists.append(np.asarray(glist, np.int64))

        rv = np.zeros(W, np.float32)
        n = min(W, M - lo)
        rv[:n] = rec_full[lo:lo + n]
        rbs.append(np.ascontiguousarray(
            np.broadcast_to(rv.astype(ml_dtypes.bfloat16), (128, W))))

    in_maps = [None] * N_CORES
    for b in range(B):
        xb = x[b, :SEN, :].astype(ml_dtypes.bfloat16)
        for q in range(NQ):
            xp = np.zeros((GP, D), ml_dtypes.bfloat16)
            xp[:SEN] = xb
            gl = glists[q]
            if gl.size:
                xp[PAD0:PAD0 + gl.size] = xb[gl]
            in_maps[b * NQ + q] = {"xh": _pm(xp), "adj": adjs[q],
                                   "rb": rbs[q]}
    return in_maps


def kernel(grid_node_features, edge_index):
    from concourse.bass_utils import run_bass_kernel_spmd

    nc = _get_nc()
    in_maps = _host_shard(grid_node_features, edge_index)
    res = run_bass_kernel_spmd(nc, in_maps, core_ids=list(range(N_CORES)))

    out = np.empty((B, M, D), dtype=np.float32)
    for c in range(N_CORES):
        b, q = divmod(c, NQ)
        lo = q * W
        cq = min(W, M - lo)
        out[b, lo:lo + cq, :] = res.results[c]["out"][:, :cq].T.astype(
            np.float32)
    return out
